# revision 3
# baseline (speedup 1.0000x reference)
"""v3: three-stage butterfly kernel, all matmuls dense on the PE.

Factor B = Bh @ Bl:
  Bl = stages 0..6  — block-diagonal over 8 contiguous 128-position blocks.
  Bh = stages 7..9  — mixes w = pos//128 across the 8 blocks, elementwise in
                      r = pos % 128.  Write r = 16*mj + ri (mj in 0..8, ri in 0..16).

Per 128-row batch chunk c:
  Stage A (PE, 8 matmuls N=128): psA[b, 128w + r] = sum_k x[b,128w+k] Bl_w[r,k]
     lhsT = x^T block [k, b], rhs = Bl_w^T [k, r].  Output orientation [b, pos].
  evictA (ACT): psA -> ysb bf16, permuted mj-major: ysb[b, 128mj + 16w + ri].
  Stage T (PE, 8 transposes): T[mj][p''=16w+ri, b] = ysb[b, 128mj + 16w + ri]
     Contiguous [128,128] input slices -> FWL-eligible stationary loads.
  evictT (DVE): psT bf16 -> tsb.
  Stage P2 (PE, 8 matmuls N=128, K=128 covers all 8 w at once):
     ps2[b, 128mj + 16wo + ri] = sum_{p''} tsb[mj][p'', b] D[mj][p'', 16wo+ri]
     D[mj][16wi+ri, 16wo+ri] = Bh[128wo + 16mj + ri, 128wi + 16mj + ri].
  evict2 (DVE): ps2 + bias -> outsb bf16 in natural [b, pos] order.
  DMA out (bf16; host upcasts to fp32).

PE: 24 matmuls x ~134cyc per chunk = ~45us/core; HBM: 8 MiB in + 8 MiB out
= ~45us/core at 358 GB/s.  Both at the roofline for this factorization.
"""

import os
import sys
import numpy as np

for _p in ("/opt/trn_rl_repo", os.path.expanduser("~/.axon_site/_ro/trn_rl_repo")):
    if os.path.isdir(_p) and _p not in sys.path:
        sys.path.insert(0, _p)

import concourse.bass as bass
import concourse.bacc as bacc
import concourse.mybir as mybir
from concourse import tile, masks
from concourse.bass_utils import run_bass_kernel_spmd

import ml_dtypes

N_CORES = 8
BATCH = 32768
N = 1024
BC = BATCH // N_CORES   # 4096 rows per core
NCHUNK = BC // 128      # 32 batch chunks per core

_last_exec_time_ns = None
_nc_cache = None


def _apply_stages(m: np.ndarray, twiddle: np.ndarray, idxs) -> np.ndarray:
    """Apply butterfly stages `idxs` to the rows of m (batch of vectors)."""
    for idx in idxs:
        s = 1 << idx
        g = N // (2 * s)
        t = twiddle[0, 0, idx].astype(np.float64).reshape(g, s, 2, 2)
        xr = m.reshape(-1, g, 2, s)
        m = np.einsum("grij,bgjr->bgir", t, xr).reshape(-1, N)
    return m


def _host_weights(twiddle: np.ndarray):
    eye = np.eye(N, dtype=np.float64)
    blt = _apply_stages(eye, twiddle, range(7))        # blt[k, p] = Bl[p, k]
    bht = _apply_stages(eye, twiddle, range(7, 10))    # bht[k, p] = Bh[p, k]

    # pass-1 rhs: bltb[k, w, r] = Bl[128w + r, 128w + k]
    bltb = np.zeros((128, 8, 128), dtype=np.float64)
    for w in range(8):
        bltb[:, w, :] = blt[128 * w:128 * (w + 1), 128 * w:128 * (w + 1)]

    # pass-2 rhs: dd2[p''=16wi+ri, mj, q=16wo+ri] = Bh[128wo+16mj+ri, 128wi+16mj+ri]
    dd2 = np.zeros((128, 8, 128), dtype=np.float64)
    ri = np.arange(16)
    for mj in range(8):
        for wi in range(8):
            for wo in range(8):
                dd2[16 * wi + ri, mj, 16 * wo + ri] = bht[
                    128 * wi + 16 * mj + ri, 128 * wo + 16 * mj + ri
                ]
    return bltb, dd2


def _build_nc():
    nc = bacc.Bacc("TRN2", target_bir_lowering=False)
    xtb = nc.dram_tensor("xtb", [128, 8, BC], mybir.dt.bfloat16, kind="ExternalInput")
    bl = nc.dram_tensor("bl", [128, 8, 128], mybir.dt.bfloat16, kind="ExternalInput")
    dd = nc.dram_tensor("dd", [128, 8, 128], mybir.dt.bfloat16, kind="ExternalInput")
    bb = nc.dram_tensor("bb", [128, N], mybir.dt.float32, kind="ExternalInput")
    out = nc.dram_tensor("out", [BC, N], mybir.dt.bfloat16, kind="ExternalOutput")

    with tile.TileContext(nc) as tc:
        with (
            tc.tile_pool(name="const", bufs=1) as cpool,
            tc.tile_pool(name="ysb", bufs=3) as y_pool,
            tc.tile_pool(name="tsb", bufs=3) as t_pool,
            tc.tile_pool(name="osb", bufs=3) as o_pool,
            tc.tile_pool(name="psA", bufs=2, space="PSUM") as psA_pool,
            tc.tile_pool(name="psT", bufs=2, space="PSUM") as psT_pool,
            tc.tile_pool(name="ps2", bufs=1, space="PSUM") as ps2_pool,
        ):
            bls = cpool.tile([128, 8, 128], mybir.dt.bfloat16)
            nc.sync.dma_start(out=bls[:], in_=bl[:])
            dds = cpool.tile([128, 8, 128], mybir.dt.bfloat16)
            nc.sync.dma_start(out=dds[:], in_=dd[:])

            ident = cpool.tile([128, 128], mybir.dt.bfloat16)
            masks.make_identity(nc, ident[:])

            xall = cpool.tile([128, 8, BC], mybir.dt.bfloat16)
            # first 512 batch cols gate the pipeline start; load them first
            nc.sync.dma_start(out=xall[:, :, 0:512], in_=xtb[:, :, 0:512])

            bbt = cpool.tile([128, N], mybir.dt.float32)
            nc.sync.dma_start(out=bbt[:], in_=bb[:])

            for g in range(1, 8):
                nc.sync.dma_start(
                    out=xall[:, :, g * 512:(g + 1) * 512],
                    in_=xtb[:, :, g * 512:(g + 1) * 512],
                )

            def stage1(c):
                bsl = slice(c * 128, (c + 1) * 128)
                psA = psA_pool.tile([128, N], mybir.dt.float32)
                for w in range(8):
                    nc.tensor.matmul(
                        psA[:, 128 * w:128 * (w + 1)],
                        xall[:, w, bsl],
                        bls[:, w, :],
                        start=True,
                        stop=True,
                    )
                ysb = y_pool.tile([128, N], mybir.dt.bfloat16)
                # ysb[b, 128mj + 16w + ri] = psA[b, 128w + 16mj + ri]
                nc.scalar.copy(
                    out=ysb[:].rearrange("p (mj w ri) -> p w mj ri", mj=8, w=8, ri=16),
                    in_=psA[:].rearrange("p (w mj ri) -> p w mj ri", w=8, mj=8, ri=16),
                )
                return ysb

            def stage2(c, ysb):
                psT = psT_pool.tile([128, N], mybir.dt.bfloat16)
                for mj in range(8):
                    nc.tensor.transpose(
                        psT[:, 128 * mj:128 * (mj + 1)],
                        ysb[:, 128 * mj:128 * (mj + 1)],
                        ident[:],
                    )
                tsb = t_pool.tile([128, 8, 128], mybir.dt.bfloat16)
                nc.vector.tensor_copy(out=tsb[:], in_=psT[:])

                ps2 = ps2_pool.tile([128, N], mybir.dt.float32)
                for mj in range(8):
                    nc.tensor.matmul(
                        ps2[:, 128 * mj:128 * (mj + 1)],
                        tsb[:, mj, :],
                        dds[:, mj, :],
                        start=True,
                        stop=True,
                    )
                outsb = o_pool.tile([128, N], mybir.dt.bfloat16)
                # out[b, 128wo + 16mj + ri] = ps2[b, 128mj + 16wo + ri] + bias
                nc.vector.tensor_add(
                    outsb[:].rearrange("p (wo mj ri) -> p mj wo ri", wo=8, mj=8, ri=16),
                    ps2[:].rearrange("p (mj wo ri) -> p mj wo ri", mj=8, wo=8, ri=16),
                    bbt[:].rearrange("p (wo mj ri) -> p mj wo ri", wo=8, mj=8, ri=16),
                )
                row0 = c * 128
                nc.scalar.dma_start(out=out[row0:row0 + 128, :], in_=outsb[:])

            # one-chunk software pipeline: stage1(c+1) is emitted before
            # stage2(c) so the PE never sits idle behind an eviction
            prev = None
            for c in range(NCHUNK):
                ysb = stage1(c)
                if prev is not None:
                    stage2(c - 1, prev)
                prev = ysb
            stage2(NCHUNK - 1, prev)

    nc.compile()
    return nc


def kernel(x: np.ndarray, twiddle: np.ndarray, bias: np.ndarray) -> np.ndarray:
    global _last_exec_time_ns, _nc_cache

    bltb, dd2 = _host_weights(twiddle)
    bl_host = np.ascontiguousarray(bltb.astype(ml_dtypes.bfloat16))
    dd_host = np.ascontiguousarray(dd2.astype(ml_dtypes.bfloat16))
    bb_host = np.ascontiguousarray(
        np.broadcast_to(np.asarray(bias, dtype=np.float32), (128, N))
    )

    x = np.ascontiguousarray(x, dtype=np.float32)
    xb = x.astype(ml_dtypes.bfloat16)
    xtb_all = np.ascontiguousarray(
        xb.reshape(N_CORES, BC, 8, 128).transpose(0, 3, 2, 1)
    )

    if _nc_cache is None:
        _nc_cache = _build_nc()
    nc = _nc_cache

    in_maps = [
        {"xtb": xtb_all[i], "bl": bl_host, "dd": dd_host, "bb": bb_host}
        for i in range(N_CORES)
    ]

    trace = bool(int(os.environ.get("BUTTERFLY_TRACE", "0")))
    res = run_bass_kernel_spmd(
        nc,
        in_maps,
        core_ids=list(range(N_CORES)),
        trace=trace,
    )
    _last_exec_time_ns = res.exec_time_ns

    return np.concatenate(
        [res.results[i]["out"].astype(np.float32) for i in range(N_CORES)], axis=0
    )
